# revision 30
# baseline (speedup 1.0000x reference)
"""Trainium2 Bass kernel for nn_PolicyNetwork3 (2-layer GraphSAGE + edge-MLP).

Design (8 NeuronCores, SPMD single NEFF):
- dst-sharded aggregation: core k owns node block [6272k, 6272k+6272).
- Node tables stored PIECE-MAJOR in one concatenated [50176, 128] table;
  the inter-layer AllGather streams piece-wise into row slices of it,
  overlapped with the window loop. Gather indices are SIGNED int16 offsets
  from the table midpoint (the Q7 address math sign-extends), so one call
  covers the whole table.
- Edges bucketed per 128-dst window; ONE dma_gather call per window
  (amortizes the ~3us per-call Q7 overhead). Every call's last index is
  kept non-negative so the ucode's trailing-negative trim never fires.
- segment-sum per window via one-hot matmuls accumulating in PSUM
  (one-hot weighted by 1/max(deg,1), built on DVE).
- Linear+BN (folded) fused per window, flipped to produce [feat, node];
  layer 1 fuses the candidate-MLP first-layer projection (gq tables).
- Candidate MLP runs in 512-candidate tiles pipelined with the u/v
  gathers; global softmax on-device after an AllGather of y.
"""

import sys

sys.path.insert(0, "/opt/trn_rl_repo")
sys.path.insert(0, "/root/.axon_site")

import numpy as np

import concourse.bacc as bacc
import concourse.bass as bass
import concourse.bass_isa as bass_isa
import concourse.mybir as mybir
import concourse.tile as tile
from concourse import library_config
from concourse.bass_utils import run_bass_kernel_spmd

P = 128
N, E, C = 50000, 800000, 100000
D = 128
NCORE = 8
NSH = 6272          # nodes per core shard (49 blocks of 128)
NTOT = NSH * NCORE  # 50176 padded node table
NWIN = NSH // P     # 49 windows (= 128-node blocks)
WPW = 128           # aggregation window width (1 block)
NPAIR = -(-NSH // WPW)  # 25 window-pairs (24 full + 1 half)
HALF = NTOT // 2    # gather base row (signed idx16 offsets from here)
CSH = C // NCORE    # 12500 candidates per core
GMAX = 2560         # max idxs per dma_gather call
TGRP = 4            # candidate chunks per MLP tile (512 cands)
NPIECE = 5
PIECE_BLK = [13, 13, 13, 8, 2]        # near-equal early pieces; short last
                                      # two so the end-of-layer AllGather
                                      # tail is mostly hidden
PIECE_R0 = [0, 1664, 3328, 4992, 6016]
PIECE_ROWS = [1664, 1664, 1664, 1024, 256]
PIECE_CATB = [0, 13312, 26624, 39936, 48128]  # piece base row in cat table
BN_EPS = 1e-5
SLOPE = 0.01
F32 = mybir.dt.float32
BF16 = mybir.dt.bfloat16
I16 = mybir.dt.int16
AF = mybir.ActivationFunctionType
ALU = mybir.AluOpType


def _wrap16(idx_lin):
    """[n] -> [128, n/16] int16 in the dma_gather wrapped+replicated layout."""
    n = idx_lin.shape[0]
    assert n % 16 == 0
    w = idx_lin.reshape(n // 16, 16).T.astype(np.int16)
    return np.tile(w, (8, 1)).copy()


def gidx_to_cols(arr):
    """[nslot] -> [128, nchunk] with slot i at [i%128, i//128]."""
    n = arr.shape[0]
    return arr.reshape(n // P, P).T.copy()


def _cat_row(src):
    """global node id -> row in the piece-major cat table."""
    src = np.asarray(src)
    k = src // NSH
    r = src % NSH
    p = np.searchsorted(np.asarray(PIECE_R0[1:]), r, side="right")
    return (np.asarray(PIECE_CATB)[p] + k * np.asarray(PIECE_ROWS)[p]
            + (r - np.asarray(PIECE_R0)[p]))


def _split_calls(p0, p1):
    """slot range -> near-equal calls of <= GMAX idxs (multiples of 128)."""
    total = p1 - p0
    ncall = -(-total // GMAX)
    per = -(-(total // P) // ncall) * P
    calls = []
    q = p0
    while q < p1:
        n = min(per, p1 - q)
        calls.append((q, n))
        q += n
    return calls


def _guard_calls(idxmats, calls):
    """Ensure the last slot of every call has all idx matrices >= 0 there,
    swapping in a suitable slot from the same call (or whole range)."""
    nmat = len(idxmats)
    ok = np.ones(idxmats[0].shape[0], bool)
    for m in idxmats:
        ok &= (m >= 0)
    lasts = {q + n - 1 for (q, n) in calls}
    for (q, n) in calls:
        last = q + n - 1
        if ok[last]:
            continue
        cand = [j for j in range(q, q + n - 1) if ok[j] and j not in lasts]
        assert cand, "no non-negative slot available for call guard"
        j = cand[-1]
        for m in idxmats:
            m[last], m[j] = m[j], m[last]
    return idxmats


def _prep_edges(src, dst, invdeg):
    """Per-window buckets of 128-edge chunks, uniform across cores."""
    core = np.minimum(dst // NSH, NCORE - 1)
    local = dst - core * NSH
    winl = local // WPW
    g = _cat_row(src)
    key = core * NPAIR + winl
    nbkt = NCORE * NPAIR
    order = np.argsort(key, kind="stable")
    cnt = np.bincount(key, minlength=nbkt).reshape(NCORE, NPAIR)
    nch_u = (-(-cnt // P)).max(axis=0)       # [NPAIR] uniform chunk counts
    run_off = np.zeros(NPAIR + 1, np.int64)
    np.cumsum(nch_u, out=run_off[1:])
    tot_ch = int(run_off[-1])
    nslot = tot_ch * P

    gidx = np.zeros((NCORE, nslot), np.int16)        # pads read row HALF
    dstloc = np.full((NCORE, nslot), -5.0, np.float32)
    val = np.zeros((NCORE, nslot), np.float32)
    bstart = np.zeros(nbkt + 1, np.int64)
    np.cumsum(np.bincount(key, minlength=nbkt), out=bstart[1:])
    run_calls = [_split_calls(int(run_off[w]) * P, int(run_off[w + 1]) * P)
                 for w in range(NPAIR)]
    for k in range(NCORE):
        for w in range(NPAIR):
            b = k * NPAIR + w
            e0, e1 = bstart[b], bstart[b + 1]
            n = e1 - e0
            if n == 0:
                continue
            sl = order[e0:e1]
            sl = sl[np.argsort(g[sl], kind="stable")]
            pos = int(run_off[w]) * P
            gidx[k, pos : pos + n] = (g[sl] - HALF).astype(np.int16)
            dstloc[k, pos : pos + n] = (local[sl] % WPW).astype(np.float32)
            val[k, pos : pos + n] = invdeg[dst[sl]]
        # keep last slot of every call non-negative (trim guard); swap
        # dstloc/val along with the index
        for w in range(NPAIR):
            calls = run_calls[w]
            lasts = [q + n - 1 for (q, n) in calls]
            for (q, n) in calls:
                last = q + n - 1
                if gidx[k, last] >= 0:
                    continue
                p0, p1 = int(run_off[w]) * P, int(run_off[w + 1]) * P
                cand = [j for j in range(p1 - 1, p0 - 1, -1)
                        if gidx[k, j] >= 0 and j not in lasts]
                assert cand, "no non-negative index in window bucket"
                j = cand[0]
                for m in (gidx, dstloc, val):
                    m[k, last], m[k, j] = m[k, j], m[k, last]
    meta = dict(run_off=run_off, run_nch=[int(x) for x in nch_u],
                tot_ch=tot_ch, nslot=nslot, run_calls=run_calls)
    data = [dict(gidx=_wrap16(gidx[k]),
                 dstloc=gidx_to_cols(dstloc[k]),
                 val=gidx_to_cols(val[k])) for k in range(NCORE)]
    return meta, data


def _prep_cands(cand_u, cand_v, cand_feat):
    """Shard candidates per core, sort by u row, pad to uniform chunks."""
    gu = _cat_row(cand_u) - HALF
    gv = _cat_row(cand_v) - HALF
    ncc = -(-CSH // P)
    cslot = ncc * P
    cu = np.zeros((NCORE, cslot), np.int16)
    cv = np.zeros((NCORE, cslot), np.int16)
    ft = np.zeros((NCORE, cslot), np.float32)
    mask = np.full((NCORE, cslot), -1e30, np.float32)
    slotmap = np.full((NCORE, cslot), -1, np.int64)
    calls = []
    q = 0
    while q < cslot:
        n = min(2048, cslot - q)
        calls.append((q, n))
        q += n
    for k in range(NCORE):
        ids = np.arange(k * CSH, (k + 1) * CSH)
        ids = ids[np.argsort(gu[ids], kind="stable")]
        cu[k, :CSH] = gu[ids].astype(np.int16)
        cv[k, :CSH] = gv[ids].astype(np.int16)
        ft[k, :CSH] = cand_feat[ids, 0]
        mask[k, :CSH] = 0.0
        slotmap[k, :CSH] = ids
        # joint guard: last slot of each call needs cu>=0 AND cv>=0
        both = (cu[k] >= 0) & (cv[k] >= 0)
        lasts = [q + n - 1 for (q, n) in calls]
        for (q, n) in calls:
            last = q + n - 1
            if both[last]:
                continue
            cand = [j for j in range(q + n - 2, q - 1, -1)
                    if both[j] and j not in lasts]
            if not cand:
                cand = [j for j in range(cslot - 1, -1, -1)
                        if both[j] and j not in lasts]
            assert cand, "no jointly non-negative candidate for call guard"
            j = cand[0]
            for m in (cu, cv, ft, mask):
                m[k, last], m[k, j] = m[k, j], m[k, last]
            slotmap[k, last], slotmap[k, j] = slotmap[k, j], slotmap[k, last]
            both[last], both[j] = both[j], both[last]
    meta = dict(ncc=ncc, cslot=cslot, calls=calls)
    data = [dict(cu=_wrap16(cu[k]), cv=_wrap16(cv[k]),
                 feat=gidx_to_cols(ft[k]), mask=gidx_to_cols(mask[k]),
                 slotmap=slotmap[k]) for k in range(NCORE)]
    return meta, data


def _build_nc(em, cm):
    nc = bacc.Bacc("TRN2", target_bir_lowering=False, debug=False,
                   num_devices=NCORE, num_swdge_queues=4)
    f32 = F32
    TOTCH, NSLOT = em["tot_ch"], em["nslot"]
    NCC, CSLOT = cm["ncc"], cm["cslot"]

    # ---- external inputs ----
    xcat = nc.dram_tensor("xcat", [NTOT, D], BF16, kind="ExternalInput")
    xT = nc.dram_tensor("xT", [P, NSH], f32, kind="ExternalInput")
    gidx = nc.dram_tensor("gidx", [P, NSLOT // 16], I16, kind="ExternalInput")
    dstloc = nc.dram_tensor("dstloc", [P, TOTCH], f32, kind="ExternalInput")
    val = nc.dram_tensor("val", [P, TOTCH], f32, kind="ExternalInput")
    wself = [nc.dram_tensor(f"wself{l}", [D, D], f32, kind="ExternalInput") for l in range(2)]
    wneigh = [nc.dram_tensor(f"wneigh{l}", [D, D], f32, kind="ExternalInput") for l in range(2)]
    crow = [nc.dram_tensor(f"crow{l}", [1, D], f32, kind="ExternalInput") for l in range(2)]
    iota = nc.dram_tensor("iota", [P, WPW], f32, kind="ExternalInput")
    ident = nc.dram_tensor("ident", [P, P], f32, kind="ExternalInput")
    onesr = nc.dram_tensor("onesr", [1, P], f32, kind="ExternalInput")
    abmat = nc.dram_tensor("abmat", [D, D], f32, kind="ExternalInput")
    gqbias = nc.dram_tensor("gqbias", [1, D], f32, kind="ExternalInput")
    mw0r = nc.dram_tensor("mw0r", [P, 64], f32, kind="ExternalInput")
    mw1b = nc.dram_tensor("mw1b", [65, 64], f32, kind="ExternalInput")
    mw2b = nc.dram_tensor("mw2b", [65, 1], f32, kind="ExternalInput")
    cu = nc.dram_tensor("cu", [P, CSLOT // 16], I16, kind="ExternalInput")
    cv = nc.dram_tensor("cv", [P, CSLOT // 16], I16, kind="ExternalInput")
    feat = nc.dram_tensor("feat", [P, NCC], f32, kind="ExternalInput")
    maskr = nc.dram_tensor("maskr", [P, NCC], f32, kind="ExternalInput")
    # ---- outputs ----
    y_out = nc.dram_tensor("y_out", [P, NCC], f32, kind="ExternalOutput")
    p_out = nc.dram_tensor("p_out", [P, NCORE * CSLOT // P], f32, kind="ExternalOutput")
    # ---- internal DRAM ----
    hshp = [nc.dram_tensor(f"hshp{p}", [PIECE_ROWS[p], D], BF16, kind="Internal")
            for p in range(NPIECE)]
    hcat = nc.dram_tensor("hcat", [NTOT, D], BF16, kind="Internal", addr_space="Shared")
    gqshp = [nc.dram_tensor(f"gqshp{p}", [PIECE_ROWS[p], D], BF16, kind="Internal")
             for p in range(NPIECE)]
    gqcat = nc.dram_tensor("gqcat", [NTOT, D], BF16, kind="Internal", addr_space="Shared")
    ysh = nc.dram_tensor("ysh", [P, NCC], f32, kind="Internal")
    yfull = nc.dram_tensor("yfull", [NCORE * P, NCC], f32, kind="Internal", addr_space="Shared")

    rg = [list(range(NCORE))]
    run_nch, run_calls = em["run_nch"], em["run_calls"]
    # piece p's last block -> the window iteration that computes it
    pend = np.array([(int(b) - 1) // (WPW // P) for b in np.cumsum(PIECE_BLK)])

    with tile.TileContext(nc) as tc:
        with (
            tc.tile_pool(name="const", bufs=1) as cp,
            tc.tile_pool(name="big", bufs=1) as bp,
            tc.tile_pool(name="msgs", bufs=8) as mp,
            tc.tile_pool(name="oh", bufs=8) as ohp,
            tc.tile_pool(name="wrk", bufs=4) as wp,
            tc.tile_pool(name="zt", bufs=3) as zp,
            tc.tile_pool(name="ps_run", bufs=2, space="PSUM") as ps_run,
            tc.tile_pool(name="ps_t", bufs=2, space="PSUM") as ps_t,
            tc.tile_pool(name="ps_h", bufs=2, space="PSUM") as ps_h,
        ):
            nc.gpsimd.load_library(library_config.mlp)

            def load(pool, t, shape=None):
                tl = pool.tile(shape or list(t.shape), t.dtype, tag=t.name)
                nc.sync.dma_start(tl[:], t[:])
                return tl

            gidx_t = load(cp, gidx)
            dstloc_t = load(cp, dstloc)
            val_t = load(cp, val)
            iota_t = load(cp, iota)
            ident_t = load(cp, ident)
            onesr_t = load(cp, onesr)
            wself_t = [load(cp, w) for w in wself]
            wneigh_t = [load(cp, w) for w in wneigh]
            crow_t = [load(cp, w) for w in crow]
            abmat_t = load(cp, abmat)
            gqbias_t = load(cp, gqbias)
            mw0r_t = load(cp, mw0r)
            mw1b_t = load(cp, mw1b)
            mw2b_t = load(cp, mw2b)
            cu_t = load(cp, cu)
            cv_t = load(cp, cv)
            feat_t = load(cp, feat)
            mask_t = load(cp, maskr)

            curT = bp.tile([P, NSH], f32, tag="curT")
            nxtT = bp.tile([P, NSH], f32, tag="nxtT")
            nc.sync.dma_start(curT[:], xT[:])

            # zero gather bufs once: short calls leave stale tail chunks
            # whose one-hot is all-zero; 0*NaN would poison the psum
            for _ in range(3):
                g0 = mp.tile([P, GMAX // P, P], BF16, tag="g")
                nc.vector.memset(g0[:], 0.0)
            ut = bp.tile([P, NCC, D], BF16, tag="ut")
            vt = bp.tile([P, NCC, D], BF16, tag="vt")

            qrr = [0]
            for layer in range(2):
                tab = xcat if layer == 0 else hcat
                for w in range(NPAIR):
                    wch = run_nch[w]
                    aggw = wp.tile([P, WPW], f32, tag="aggw")
                    if wch == 0:
                        nc.vector.memset(aggw[:], 0.0)
                    else:
                        ps = ps_run.tile([P, WPW], f32, tag="psw")
                        jw = 0
                        for (s0, n_idx) in run_calls[w]:
                            ncall = n_idx // P
                            g = mp.tile([P, GMAX // P, P], BF16, tag="g")
                            nc.gpsimd.dma_gather(
                                g[:, :ncall, :], tab[HALF:NTOT, :],
                                gidx_t[:, s0 // 16 : (s0 + n_idx) // 16],
                                n_idx, n_idx, P, single_packet=False,
                                queue_num=qrr[0])
                            qrr[0] = (qrr[0] + 1) % 4
                            for cc in range(ncall):
                                ch = s0 // P + cc
                                oh = ohp.tile([P, WPW], BF16, tag="oh")
                                nc.vector.tensor_scalar(
                                    oh[:], iota_t[:], dstloc_t[:, ch : ch + 1],
                                    val_t[:, ch : ch + 1], ALU.is_equal, ALU.mult)
                                nc.tensor.matmul(ps[:], lhsT=g[:, cc, :], rhs=oh[:],
                                                 start=(jw == 0), stop=(jw == wch - 1))
                                jw += 1
                        nc.scalar.activation(aggw[:], ps[:], AF.Copy)
                    for bb in range(min(WPW // P, NWIN - (WPW // P) * w)):
                        nb = w * WPW + bb * P
                        ph = ps_h.tile([P, P], f32, tag="ph")
                        nc.tensor.matmul(ph[:], lhsT=wneigh_t[layer][:],
                                         rhs=aggw[:, bb * P : (bb + 1) * P],
                                         start=True, stop=False)
                        nc.tensor.matmul(ph[:], lhsT=wself_t[layer][:],
                                         rhs=curT[:, nb : nb + P], start=False, stop=False)
                        nc.tensor.matmul(ph[:], lhsT=crow_t[layer][:], rhs=onesr_t[:],
                                         start=False, stop=True)
                        tmp = wp.tile([P, P], f32, tag="tmp")
                        nc.vector.tensor_scalar(tmp[:], ph[:], SLOPE, None, ALU.mult)
                        nc.vector.tensor_tensor(nxtT[:, nb : nb + P], ph[:], tmp[:], ALU.max)
                        wp_ = next(i for i in range(NPIECE)
                                   if PIECE_R0[i] <= nb < PIECE_R0[i] + PIECE_BLK[i] * P)
                        rb = nb - PIECE_R0[wp_]
                        if layer == 0:
                            pt = ps_t.tile([P, P], f32, tag="pt")
                            nc.tensor.transpose(pt[:], nxtT[:, nb : nb + P], ident_t[:])
                            hb = wp.tile([P, P], BF16, tag="hb")
                            nc.scalar.activation(hb[:], pt[:], AF.Copy)
                            nc.sync.dma_start(hshp[wp_][rb : rb + P, :], hb[:])
                        else:
                            pg = ps_h.tile([P, P], f32, tag="ph")
                            nc.tensor.matmul(pg[:], lhsT=nxtT[:, nb : nb + P],
                                             rhs=abmat_t[:], start=True, stop=False)
                            nc.tensor.matmul(pg[:], lhsT=onesr_t[:], rhs=gqbias_t[:],
                                             start=False, stop=True)
                            gb = wp.tile([P, P], BF16, tag="gb")
                            nc.scalar.activation(gb[:], pg[:], AF.Copy)
                            nc.sync.dma_start(gqshp[wp_][rb : rb + P, :], gb[:])
                    if w in pend:
                        pi = int(np.nonzero(pend == w)[0][0])
                        cb0 = PIECE_CATB[pi]
                        cb1 = cb0 + NCORE * PIECE_ROWS[pi]
                        if layer == 0:
                            nc.gpsimd.collective_compute(
                                "AllGather", ALU.bypass, replica_groups=rg,
                                ins=[hshp[pi][:].opt()], outs=[hcat[cb0:cb1, :].opt()])
                        else:
                            nc.gpsimd.collective_compute(
                                "AllGather", ALU.bypass, replica_groups=rg,
                                ins=[gqshp[pi][:].opt()], outs=[gqcat[cb0:cb1, :].opt()])
                if layer == 0:
                    curT, nxtT = nxtT, curT

            # ---- candidate gathers (u then v, interleaved by slot) ----
            for (s0, n_idx) in cm["calls"]:
                for (tl, idx_t) in ((ut, cu_t), (vt, cv_t)):
                    nc.gpsimd.dma_gather(
                        tl[:, s0 // P : (s0 + n_idx) // P, :],
                        gqcat[HALF:NTOT, :],
                        idx_t[:, s0 // 16 : (s0 + n_idx) // 16],
                        n_idx, n_idx, D, single_packet=False,
                        queue_num=qrr[0])
                    qrr[0] = (qrr[0] + 1) % 4

            # ---- candidate MLP in 512-cand tiles, 2-stage pipelined ----
            ycol = bp.tile([P, NCC], f32, tag="ycol")

            def mlp_stage_a(t0):
                tn = min(TGRP, NCC - t0)
                z1 = zp.tile([P, TGRP, 64], f32, tag="z1")
                for c in range(tn):
                    nc.vector.tensor_scalar(z1[:, c, :], mw0r_t[:],
                                            feat_t[:, t0 + c : t0 + c + 1],
                                            None, ALU.mult)
                nc.vector.tensor_tensor(z1[:, :tn, :], z1[:, :tn, :],
                                        ut[:, t0 : t0 + tn, 0:64], ALU.add)
                nc.vector.tensor_tensor(z1[:, :tn, :], z1[:, :tn, :],
                                        vt[:, t0 : t0 + tn, 64:128], ALU.add)
                zs = zp.tile([P, TGRP, 64], f32, tag="zs")
                nc.vector.tensor_scalar(zs[:, :tn, :], z1[:, :tn, :], SLOPE,
                                        None, ALU.mult)
                nc.vector.tensor_tensor(z1[:, :tn, :], z1[:, :tn, :],
                                        zs[:, :tn, :], ALU.max)
                z1t = zp.tile([65, TGRP * P], f32, tag="z1t")
                nc.vector.memset(z1t[64:65, :], 1.0)
                for c in range(tn):
                    pt2 = ps_t.tile([64, P], f32, tag="pt")
                    nc.tensor.transpose(pt2[:], z1[:, c, :], ident_t[:])
                    nc.scalar.activation(z1t[0:64, c * P : (c + 1) * P], pt2[:], AF.Copy)
                return t0, tn, z1t

            def mlp_stage_b(st):
                t0, tn, z1t = st
                ps2 = ps_run.tile([64, TGRP * P], f32, tag="ps2")
                nc.tensor.matmul(ps2[:, : tn * P], lhsT=mw1b_t[:],
                                 rhs=z1t[:, : tn * P], start=True, stop=True)
                z2t = zp.tile([65, TGRP * P], f32, tag="z2t")
                nc.vector.memset(z2t[64:65, :], 1.0)
                nc.vector.tensor_scalar(z2t[0:64, : tn * P], ps2[:, : tn * P],
                                        SLOPE, None, ALU.mult)
                nc.vector.tensor_tensor(z2t[0:64, : tn * P], z2t[0:64, : tn * P],
                                        ps2[:, : tn * P], ALU.max)
                for c in range(tn):
                    py = ps_h.tile([P, 1], f32, tag="ph")
                    nc.tensor.matmul(py[:], lhsT=z2t[:, c * P : (c + 1) * P],
                                     rhs=mw2b_t[:], start=True, stop=True)
                    nc.scalar.activation(ycol[:, t0 + c : t0 + c + 1], py[:], AF.Copy)

            pending = None
            for t0 in range(0, NCC, TGRP):
                st = mlp_stage_a(t0)
                if pending is not None:
                    mlp_stage_b(pending)
                pending = st
            mlp_stage_b(pending)

            nc.sync.dma_start(y_out[:], ycol[:])
            ym = wp.tile([P, NCC], f32, tag="ym")
            nc.vector.tensor_tensor(ym[:], ycol[:], mask_t[:], ALU.add)
            nc.sync.dma_start(ysh[:], ym[:])
            nc.gpsimd.collective_compute(
                "AllGather", ALU.bypass, replica_groups=rg,
                ins=[ysh[:].opt()], outs=[yfull[:].opt()])
            # ---- softmax ----
            ncols = NCORE * CSLOT // P
            yf = bp.tile([P, ncols], f32, tag="yf")
            nc.sync.dma_start(yf[:], yfull[:].rearrange("a b -> (a b)")
                              .rearrange("(p c) -> p c", p=P))
            rmax = wp.tile([P, 1], f32, tag="rmax")
            nc.vector.tensor_reduce(rmax[:], yf[:], mybir.AxisListType.X, ALU.max)
            gmax = wp.tile([P, 1], f32, tag="gmax")
            nc.gpsimd.partition_all_reduce(gmax[:], rmax[:], P,
                                           bass_isa.ReduceOp.max)
            ngmax = wp.tile([P, 1], f32, tag="ngmax")
            nc.vector.tensor_scalar(ngmax[:], gmax[:], -1.0, None, ALU.mult)
            ef = bp.tile([P, ncols], f32, tag="ef")
            se = wp.tile([P, 1], f32, tag="se")
            nc.scalar.activation(ef[:], yf[:], AF.Exp, bias=ngmax[:, 0:1],
                                 accum_out=se[:])
            stot = wp.tile([P, 1], f32, tag="stot")
            nc.gpsimd.partition_all_reduce(stot[:], se[:], P, bass_isa.ReduceOp.add)
            invs = wp.tile([P, 1], f32, tag="invs")
            nc.vector.reciprocal(invs[:], stot[:])
            pf = bp.tile([P, ncols], f32, tag="pf")
            nc.vector.tensor_scalar(pf[:], ef[:], invs[:, 0:1], None, ALU.mult)
            nc.sync.dma_start(p_out[:], pf[:])
    nc.compile()
    return nc


def kernel(x, src, dst, cand_u, cand_v, cand_feat,
           w_self0, w_neigh0, b0, gamma0, beta0, rm0, rv0,
           w_self1, w_neigh1, b1, gamma1, beta1, rm1, rv1,
           mw0, mb0, mw1, mb1, mw2, mb2):
    x = np.asarray(x, np.float32)
    src = np.asarray(src, np.int64)
    dst = np.asarray(dst, np.int64)
    cand_u = np.asarray(cand_u, np.int64)
    cand_v = np.asarray(cand_v, np.int64)
    cand_feat = np.asarray(cand_feat, np.float32)

    deg = np.bincount(dst, minlength=N).astype(np.float32)
    invdeg = 1.0 / np.maximum(deg, 1.0)
    em, edata = _prep_edges(src, dst, invdeg)
    cm, cdata = _prep_cands(cand_u, cand_v, cand_feat)

    xpad = np.zeros((NTOT, D), np.float32)
    xpad[:N] = x
    iota = np.tile(np.arange(WPW, dtype=np.float32), (P, 1))
    ident = np.eye(P, dtype=np.float32)
    onesr = np.ones((1, P), np.float32)

    com = {"iota": iota, "ident": ident, "onesr": onesr}
    # piece-major concatenated x table
    import ml_dtypes
    xsh = xpad.reshape(NCORE, NSH, D)
    xcat = np.zeros((NTOT, D), ml_dtypes.bfloat16)
    for p in range(NPIECE):
        r0, rows, cb = PIECE_R0[p], PIECE_ROWS[p], PIECE_CATB[p]
        xcat[cb : cb + NCORE * rows] = xsh[:, r0 : r0 + rows, :].reshape(-1, D)
    com["xcat"] = xcat
    for l, (ws, wn, b, ga, be, rme, rve) in enumerate(
        ((w_self0, w_neigh0, b0, gamma0, beta0, rm0, rv0),
         (w_self1, w_neigh1, b1, gamma1, beta1, rm1, rv1))):
        a = (ga / np.sqrt(rve + BN_EPS)).astype(np.float32)
        com[f"wself{l}"] = (ws * a[None, :]).astype(np.float32)
        com[f"wneigh{l}"] = (wn * a[None, :]).astype(np.float32)
        com[f"crow{l}"] = (a * (b - rme) + be).astype(np.float32)[None, :]
    com["abmat"] = np.concatenate(
        [np.asarray(mw0[0:128], np.float32), np.asarray(mw0[128:256], np.float32)], axis=1)
    com["gqbias"] = np.concatenate(
        [np.zeros(64, np.float32), np.asarray(mb0, np.float32)])[None, :]
    com["mw0r"] = np.tile(np.asarray(mw0[256], np.float32), (P, 1))
    com["mw1b"] = np.concatenate(
        [np.asarray(mw1, np.float32), np.asarray(mb1, np.float32)[None, :]], axis=0)
    com["mw2b"] = np.concatenate(
        [np.asarray(mw2, np.float32),
         np.asarray(mb2, np.float32).reshape(1, 1)], axis=0)

    nc = _build_nc(em, cm)
    in_maps = []
    for k in range(NCORE):
        m = dict(com)
        m["xT"] = xpad[k * NSH : (k + 1) * NSH].T.copy()
        m["gidx"] = edata[k]["gidx"]
        m["dstloc"] = edata[k]["dstloc"]
        m["val"] = edata[k]["val"]
        m["cu"] = cdata[k]["cu"]
        m["cv"] = cdata[k]["cv"]
        m["feat"] = cdata[k]["feat"]
        m["maskr"] = cdata[k]["mask"]
        in_maps.append(m)
    import os
    trace = bool(os.environ.get("KERNEL_TRACE"))
    if trace:
        import types
        import ctypes
        if "antenv.axon_hooks" not in sys.modules:
            try:
                import antenv
                from trn_agent_boot.trn_boot import _ntff_profile_via_ctypes
                mod = types.ModuleType("antenv.axon_hooks")
                hook = [_ntff_profile_via_ctypes("/opt/axon/libaxon_pjrt.so")]
                mod.set_axon_ntff_profile_hook = lambda h: hook.__setitem__(0, h)
                mod.get_axon_ntff_profile_hook = lambda: hook[0]
                sys.modules["antenv.axon_hooks"] = mod
                antenv.axon_hooks = mod
            except Exception:
                trace = False
    res = run_bass_kernel_spmd(nc, in_maps, core_ids=list(range(NCORE)),
                               trace=trace,
                               tmpdir=os.environ.get("KERNEL_TRACE_DIR"))
    if trace and res.exec_time_ns is not None:
        print(f"HW exec time: {res.exec_time_ns} ns")
    y_all = np.zeros(C, np.float32)
    p_all = np.zeros(C, np.float32)
    ncc = cm["ncc"]
    p_lin = res.results[0]["p_out"].ravel()   # global order: k, p, c
    for k in range(NCORE):
        sm = cdata[k]["slotmap"]
        valid = sm >= 0
        j = np.nonzero(valid)[0]              # slot j = c*128 + p
        yk = res.results[k]["y_out"]          # [128, NCC] -> value at [j%128, j//128]
        y_all[sm[valid]] = yk[j % P, j // P]
        gs = k * cm["cslot"] + (j % P) * ncc + (j // P)
        p_all[sm[valid]] = p_lin[gs]
    return y_all[:, None], p_all[:, None]
